# revision 7
# baseline (speedup 1.0000x reference)
"""Causal single-head attention on 8 Trainium2 NeuronCores — v2.

Problem: x[8, 4096, 512] @ W_{Q,K,V}[512, 64] -> causal softmax attention
-> out[8, 4096, 64].  Data-parallel over batch (1 element/core).

v2 design (vs baseline):
  - Host supplies xT [512, 4096] fp16 (pre-transposed + cast) -> no on-device
    x transposes; 4MB HBM load instead of 8MB.
  - Everything on-chip is fp16 (1 PE cycle/row, FWL weight loads, 2x DVE
    modes). Wq pre-scaled by 1/(8*sqrt? no: 1/8 softmax scale) on host.
  - QK^T projection: stationary [Wk|Wq] -> qkT [128, S] (k rows 0:64,
    q rows 64:128). V computed s-tile-direct: stationary xT-chunk, moving Wv
    -> v tiles in natural [k, e] layout (no transposes).
  - exp(s - 2) split across ScalarE (exact, diagonal + ~half interior) and
    VectorE (custom 2-pass op: (poly2(s))^16 -> exp((s-2)/16), then ^16 at
    fp16) to break the ACT throughput wall.
  - Softmax denominator via ones-column in the PV stationary (v_aug[:,64]=1).
  - Output shipped un-normalized as outT [65, S] fp16 (numerator rows 0:64,
    denominator row 64); host divides + transposes (free vs device time).
  - Optional fp8e4m3 DoubleRow PV (2 k-tiles per pass) for interior pairs.
"""

import sys

sys.path.insert(0, "/opt/trn_rl_repo")
sys.path.insert(0, "/root/.axon_site/_ro/trn_rl_repo")

import numpy as np

B, S, D, E = 8, 4096, 512, 64
N_CORES = 8
EXP_SHIFT = 2.0   # exp(s' - 2): fp16 range headroom; cancels in normalize
DR_SHIFT = 3.0    # extra shift for fp8e4 DoubleRow tiles (applied via exp)

_cache = {}


def _fit_pass1(shift, denom=256.0, lo=-9.5, hi=10.5):
    """q(s) = c2 s^2 + c1 s + c0 ~ exp((s - shift)/denom), relative-minimax
    via iteratively reweighted lstsq. (q^16)^16 = exp(s - shift)."""
    s = np.linspace(lo, hi, 4001)
    t = np.exp((s - shift) / denom)
    w = np.ones_like(s)
    V = np.stack([s**2, s, np.ones_like(s)], axis=1)
    c = None
    for _ in range(80):
        W = w / t
        c, *_ = np.linalg.lstsq(V * W[:, None], t * W, rcond=None)
        e = np.abs((V @ c) / t - 1)
        w *= (1e-12 + e / e.max()) ** 0.35 + 0.2
        w /= w.mean()
    return [float(v) for v in c]  # [c2, c1, c0]


def _register_dve_ops():
    """Register the 2-pass exp ops into concourse.dve_ops (idempotent)."""
    import concourse.dve_ops as dmod
    from concourse.dve_spec import Spec, Src0, C0, C1, C2, sq
    from concourse.dve_spec import lower as dve_lower
    from concourse.dve_uop import DveOpSpec

    if "ANT_EXPQ16" in dmod._SUB_OPCODE_FOR_NAME:
        by_name = {op.name: op for op in dmod.OPS}
        return by_name["ANT_EXPQ16"], by_name["ANT_POW16"]

    def make(name, spec):
        row = dmod._CUSTOM_DVE_ROW_BASE + len(dmod.OPS)
        shas = {}
        for ver in ("v3", "v4"):
            try:
                uops = dve_lower(spec, ver=ver)
                shas[ver] = DveOpSpec(
                    name=name, opcode=row, uops=uops, rd1_en=False
                ).sha(ver)
            except Exception:
                pass
        op = dmod.DveOp(name, spec, subdim=False, uops_sha=shas)
        dmod.OPS.append(op)
        dmod._SUB_OPCODE_FOR_NAME[name] = row
        dmod.CUSTOM_DVE_SPECS[name] = spec
        return op

    def ref1(in0, in1, s0, s1, imm2):
        q = (np.float32(imm2) * in0.astype(np.float32)
             + np.float32(s1)) * in0.astype(np.float32) + np.float32(s0)
        q = q.astype(np.float32)
        for _ in range(4):
            q = (q * q).astype(np.float32)
        return q

    def ref2(in0, in1, s0, s1, imm2):
        q = in0.astype(np.float32)
        for _ in range(4):
            q = (q * q).astype(np.float32)
        return q

    p = (C2 * Src0 + C1) * Src0 + C0
    expq = make("ANT_EXPQ16", Spec(body=sq(sq(sq(sq(p)))), reference=ref1))
    pow16 = make("ANT_POW16", Spec(body=sq(sq(sq(sq(Src0)))), reference=ref2))
    return expq, pow16


def _build(S=S, reps=1, dve_mod=3, dve_rems=(1,), use_dr=False,
           bufs_st=3, bufs_proj=1, bufs_o=1, evac_dve=False,
           dve_diag_cmin=2, prefetch=2, xin_bufs=4,
           ptp_bufs=4, yp_bufs=3, evac_qk_dve=True):
    import concourse.bass as bass  # noqa: F401
    import concourse.mybir as mybir
    import concourse.tile as tile
    from concourse import bacc
    from contextlib import ExitStack

    F32 = mybir.dt.float32
    F16 = mybir.dt.float16
    FP8 = mybir.dt.float8e4
    EXP = mybir.ActivationFunctionType.Exp
    DRMODE = mybir.MatmulPerfMode.DoubleRow

    EXPQ16, POW16 = _register_dve_ops()
    pc2, pc1, pc0 = _fit_pass1(EXP_SHIFT)
    pc2d, pc1d, pc0d = _fit_pass1(EXP_SHIFT + DR_SHIFT)

    T = S // 128   # k tiles
    C = S // 512   # q chunks
    DC = D // 128  # contraction chunks
    VPAD = 80      # fp8 v_aug padded width (stride % 16 == 0)

    nc = bacc.Bacc("TRN2", target_bir_lowering=False, debug=False,
                   num_devices=N_CORES)
    xt = nc.dram_tensor("xT", [D, S], F16, kind="ExternalInput").ap()
    wkq = nc.dram_tensor("WKQ", [128, DC * 128], F16, kind="ExternalInput").ap()
    wv = nc.dram_tensor("WV", [128, DC * E], F16, kind="ExternalInput").ap()
    outT = nc.dram_tensor("outT", [E + 1, S], F16, kind="ExternalOutput").ap()

    with tile.TileContext(nc) as tc:
        with ExitStack() as ctx:
            const = ctx.enter_context(tc.tile_pool(name="const", bufs=1))
            big = ctx.enter_context(tc.tile_pool(name="big", bufs=1))
            xin = ctx.enter_context(tc.tile_pool(name="xin", bufs=xin_bufs))
            ptp = ctx.enter_context(tc.tile_pool(name="ptp", bufs=ptp_bufs))
            yp = ctx.enter_context(tc.tile_pool(name="yp", bufs=yp_bufs))
            psproj = ctx.enter_context(
                tc.tile_pool(name="psp", bufs=bufs_proj, space="PSUM"))
            psst = ctx.enter_context(
                tc.tile_pool(name="psst", bufs=bufs_st, space="PSUM"))
            pso = ctx.enter_context(
                tc.tile_pool(name="pso", bufs=bufs_o, space="PSUM"))

            # ---------------- weights ----------------
            wkq_sb = const.tile([128, DC, 128], F16)
            nc.sync.dma_start(wkq_sb[:], wkq.rearrange("p (c m) -> p c m", c=DC))
            wv_sb = const.tile([128, DC, E], F16)
            nc.sync.dma_start(wv_sb[:], wv.rearrange("p (c e) -> p c e", c=DC))
            bias_a = const.tile([128, 1], F32)
            nc.gpsimd.memset(bias_a[:], -EXP_SHIFT)
            bias_d = const.tile([128, 1], F32)
            nc.gpsimd.memset(bias_d[:], -(EXP_SHIFT + DR_SHIFT))

            # ---------------- residents ----------------
            qkT = big.tile([128, S], F16)    # [0:64]=kT, [64:128]=qT
            qdup = big.tile([128, S], F16)   # [0:64]=qT dup; [64:128,:S//2]=kT odd
            v_aug = big.tile([128, T, E + 1], F16)
            nc.gpsimd.memset(v_aug[:], 1.0)  # col E stays 1.0 (denominator)
            if use_dr:
                v8 = big.tile([128, T, VPAD], FP8)
                nc.gpsimd.memset(v8[:], 0.0)
                nc.gpsimd.memset(v8[:, :, E:E + 1], 1.0)
            outsb = big.tile([E + 1, S], F16)

            xt_r = xt.rearrange("(c p) s -> p c s", p=128)

            for _rep in range(reps):
                xpre = {}

                def load_x(c):
                    xtile = xin.tile([128, DC, 512], F16, tag="xin")
                    nc.sync.dma_start(
                        xtile[:], xt_r[:, :, 512 * c:512 * (c + 1)])
                    return xtile

                def proj(c):
                    xtile = xpre.pop(c) if c in xpre else load_x(c)
                    ps_qk = psproj.tile([128, 512], F32, tag="psp")
                    for d in range(DC):
                        nc.tensor.matmul(
                            ps_qk[:], wkq_sb[:, d, :], xtile[:, d, :],
                            start=(d == 0), stop=(d == DC - 1))
                    if evac_qk_dve:
                        nc.vector.tensor_copy(
                            qkT[:, 512 * c:512 * (c + 1)], ps_qk[:])
                    else:
                        nc.scalar.copy(
                            qkT[:, 512 * c:512 * (c + 1)], ps_qk[:])
                    # duplicate qT into low partitions, odd kT tiles into high
                    nc.sync.dma_start(
                        qdup[0:64, 512 * c:512 * (c + 1)],
                        qkT[64:128, 512 * c:512 * (c + 1)])
                    odd_src = qkT[0:64, 512 * c:512 * (c + 1)].rearrange(
                        "p (a b f) -> p a b f", b=2, f=128)[:, :, 1, :]
                    nc.sync.dma_start(
                        qdup[64:128, 256 * c:256 * (c + 1)].rearrange(
                            "p (a f) -> p a f", f=128),
                        odd_src)
                    # V: four s-tiles, natural [k, e] layout
                    ps_v = psproj.tile([128, 4, E], F32, tag="psp")
                    for st in range(4):
                        for d in range(DC):
                            nc.tensor.matmul(
                                ps_v[:, st, :],
                                xtile[:, d, 128 * st:128 * (st + 1)],
                                wv_sb[:, d, :],
                                start=(d == 0), stop=(d == DC - 1))
                    nc.vector.tensor_copy(
                        v_aug[:, 4 * c:4 * c + 4, 0:E], ps_v[:])
                    if use_dr:
                        nc.vector.tensor_copy(
                            v8[:, 4 * c:4 * c + 4, 0:E], ps_v[:])

                def scores(c, j):
                    t0 = 2 * j
                    d0 = 128 * t0 - 512 * c
                    c0, c1_ = max(d0, 0), max(d0 + 128, 0)
                    ps_pair = psst.tile([128, 1024], F32, tag="st")
                    nc.tensor.matmul(
                        ps_pair[:, c0:512],
                        qkT[0:64, 128 * t0:128 * (t0 + 1)],
                        qdup[0:64, 512 * c + c0:512 * (c + 1)],
                        start=True, stop=True, tile_position=(0, 0))
                    nc.tensor.matmul(
                        ps_pair[:, 512 + c1_:1024],
                        qdup[64:128, 128 * j:128 * (j + 1)],
                        qkT[64:128, 512 * c + c1_:512 * (c + 1)],
                        start=True, stop=True, tile_position=(64, 0))
                    return ps_pair

                def attn(c):
                    npair = 2 * c + 2
                    ps_o = pso.tile([VPAD if use_dr else E + 1, 512], F32,
                                    tag="pso")
                    pair_q = [scores(c, 0)]
                    for j in range(npair):
                        if j + 1 < npair:
                            pair_q.append(scores(c, j + 1))
                        ps_pair = pair_q[j]
                        t0, t1 = 2 * j, 2 * j + 1
                        d0 = 128 * t0 - 512 * c
                        d1 = d0 + 128
                        c0, c1_ = max(d0, 0), max(d1, 0)
                        diag = d0 >= 0
                        dve = ((j % dve_mod) in dve_rems) and (
                            (not diag) or c >= dve_diag_cmin)
                        dr = use_dr and not diag
                        first, last = (j == 0), (j == npair - 1)
                        if dr:
                            pt8 = ptp.tile([128, 1024], FP8, tag="pt8")
                            pt2 = pt8[:].rearrange("p (a b) -> p a b", a=2)
                            if dve:
                                y = yp.tile([128, 1024], F16, tag="y")
                                nc.vector._custom_dve(
                                    EXPQ16, out=y[:], in0=ps_pair[:],
                                    s0=pc0d, s1=pc1d, imm2=pc2d)
                                nc.vector._custom_dve(
                                    POW16, out=pt8[:], in0=y[:])
                            else:
                                nc.scalar.activation(
                                    pt8[:], ps_pair[:], EXP,
                                    bias=bias_d[:])
                            nc.tensor.matmul(
                                ps_o[:, 0:512], v8[:, t0:t0 + 2, :], pt2,
                                start=first, stop=last, perf_mode=DRMODE)
                        else:
                            pt = ptp.tile([128, 1024], F16, tag="pt")
                            if dve and c1_ == 0:
                                y = yp.tile([128, 1024], F16, tag="y")
                                nc.vector._custom_dve(
                                    EXPQ16, out=y[:], in0=ps_pair[:],
                                    s0=pc0, s1=pc1, imm2=pc2)
                                nc.vector._custom_dve(
                                    POW16, out=pt[:], in0=y[:])
                            elif dve:
                                y = yp.tile([128, 1024], F16, tag="y")
                                for r0, r1 in ((c0, 512), (512 + c1_, 1024)):
                                    nc.vector._custom_dve(
                                        EXPQ16, out=y[:, r0:r1],
                                        in0=ps_pair[:, r0:r1],
                                        s0=pc0, s1=pc1, imm2=pc2)
                                    nc.vector._custom_dve(
                                        POW16, out=pt[:, r0:r1],
                                        in0=y[:, r0:r1])
                            elif c1_ == 0:
                                nc.scalar.activation(
                                    pt[:, 0:1024], ps_pair[:, 0:1024], EXP,
                                    bias=bias_a[:])
                            else:
                                nc.scalar.activation(
                                    pt[:, c0:512], ps_pair[:, c0:512], EXP,
                                    bias=bias_a[:])
                                nc.scalar.activation(
                                    pt[:, 512 + c1_:1024],
                                    ps_pair[:, 512 + c1_:1024], EXP,
                                    bias=bias_a[:])
                            if 0 <= d0:
                                nc.gpsimd.affine_select(
                                    out=pt[:, d0:d0 + 128],
                                    in_=pt[:, d0:d0 + 128],
                                    compare_op=mybir.AluOpType.is_ge,
                                    fill=0.0, base=0, pattern=[[1, 128]],
                                    channel_multiplier=-1)
                            if 0 <= d1 < 512:
                                nc.gpsimd.affine_select(
                                    out=pt[:, 512 + d1:512 + d1 + 128],
                                    in_=pt[:, 512 + d1:512 + d1 + 128],
                                    compare_op=mybir.AluOpType.is_ge,
                                    fill=0.0, base=0, pattern=[[1, 128]],
                                    channel_multiplier=-1)
                            nc.tensor.matmul(
                                ps_o[0:E + 1, c0:512], v_aug[:, t0, :],
                                pt[:, c0:512], start=first, stop=False)
                            nc.tensor.matmul(
                                ps_o[0:E + 1, c1_:512], v_aug[:, t1, :],
                                pt[:, 512 + c1_:1024],
                                start=False, stop=last)
                    # epilogue: un-normalized numerator+denominator, fp16
                    if evac_dve:
                        nc.vector.tensor_copy(
                            outsb[:, 512 * c:512 * (c + 1)], ps_o[0:E + 1, :])
                    else:
                        nc.scalar.copy(
                            outsb[:, 512 * c:512 * (c + 1)], ps_o[0:E + 1, :])
                    nc.sync.dma_start(
                        outT[:, 512 * c:512 * (c + 1)],
                        outsb[:, 512 * c:512 * (c + 1)])

                for c in range(min(prefetch, C)):
                    xpre[c] = load_x(c)
                proj(0)
                for c in range(C):
                    if c + 1 < C:
                        proj(c + 1)
                    attn(c)

    nc.compile()
    return nc


def _get_nc():
    if "nc" not in _cache:
        _cache["nc"] = _build()
    return _cache["nc"]


def _prep_inputs(x, W_Q, W_K, W_V):
    """Host-side packing: xT fp16, packed fp16 weights (Wq pre-scaled)."""
    x = np.asarray(x, dtype=np.float32)
    wq = np.asarray(W_Q, dtype=np.float32) / 8.0   # softmax 1/sqrt(64)
    wk = np.asarray(W_K, dtype=np.float32)
    wv = np.asarray(W_V, dtype=np.float32)
    # wkq[p, c*128 + m]: W[c*128 + p, col] with cols [0:64]=Wk, [64:128]=Wq
    wkq = np.concatenate([wk, wq], axis=1)          # [512, 128]
    wkq = wkq.reshape(4, 128, 128).transpose(1, 0, 2).reshape(128, 512)
    wvp = wv.reshape(4, 128, E).transpose(1, 0, 2).reshape(128, 4 * E)
    xts = [np.ascontiguousarray(x[b].T).astype(np.float16) for b in range(B)]
    return xts, wkq.astype(np.float16), wvp.astype(np.float16)


def kernel(x, W_Q, W_K, W_V):
    from concourse import bass_utils

    xts, wkq, wvp = _prep_inputs(x, W_Q, W_K, W_V)
    nc = _get_nc()
    in_maps = [
        {"xT": xts[b], "WKQ": wkq, "WV": wvp} for b in range(B)
    ]
    res = bass_utils.run_bass_kernel_spmd(nc, in_maps,
                                          core_ids=list(range(N_CORES)))
    out = np.empty((B, S, E), dtype=np.float32)
    for b in range(B):
        ot = np.asarray(res.results[b]["outT"], dtype=np.float32)  # [65, S]
        out[b] = (ot[0:E] / ot[E]).T
    return out
